# revision 58
# baseline (speedup 1.0000x reference)
"""Trainium2 Bass kernel for nn_EmotionalEmbeddingSpace.

Sharding: data-parallel over batch B=16 across 8 cores (2 sequences/core).
Layout on device: transposed — features on partitions, tokens on the free dim.

The serial memory recurrence (1023 steps of tanh(pt_j + Um^T mem_{j-1}))
dominates: each step is 37 small matmuls (LDWEIGHTS-bound) + one ACT tanh.
To keep the PE array continuously busy (avoiding p-state/HAM throttling and
hiding all MLP work), every other phase — encode(x), decode+recon+trans,
encode(mem), ctx — is emitted as fine-grained "filler" ops interleaved into
the recurrence's per-step idle windows by a build-time scheduler.

Matmul inputs are bf16 (PSUM accumulation in f32); LN statistics bf16/f32
mixed; loss math f32.
"""

import sys

sys.path.insert(0, "/opt/trn_rl_repo")

import numpy as np
import ml_dtypes

import concourse.bass as bass
import concourse.bacc as bacc
import concourse.mybir as mybir
import concourse.tile as tile
import concourse.bass_utils as _bass_utils
from concourse.bass_utils import run_bass_kernel_spmd

import os as _os

F32 = mybir.dt.float32
BF16 = mybir.dt.bfloat16
FP8 = mybir.dt.float8e4
AF = mybir.ActivationFunctionType
ALU = mybir.AluOpType

B, S_FULL, D, H, L = 16, 1024, 768, 512, 128
NCORES = 8
LN_EPS = 1e-5
NORM_EPS = 1e-8


# ---------------------------------------------------------------- host prep

def _pack_cols(*vecs):
    cols = []
    for v in vecs:
        v = np.asarray(v, np.float32).reshape(-1, 128)
        cols.append(v.T)
    return np.ascontiguousarray(np.concatenate(cols, axis=1))


def _ln_np(x, g, b, eps=LN_EPS):
    m = x.mean(-1, keepdims=True)
    v = ((x - m) ** 2).mean(-1, keepdims=True)
    return (x - m) / np.sqrt(v + eps) * g + b


def _encode_np(t, w):
    h = np.maximum(_ln_np(t @ w["W1"] + w["b1"], w["g1"], w["be1"]), 0)
    a = h @ w["Wvo"] + w["bvo"]
    g = np.maximum(_ln_np(a @ w["W2"] + w["b2"], w["g2"], w["be2"]), 0)
    zl = _ln_np(g @ w["W3"] + w["b3"], w["g3"], w["be3"])
    e = np.maximum(_ln_np(zl @ w["W4"] + w["b4"], w["g4"], w["be4"]), 0)
    return _ln_np(e @ w["W5"] + w["b5"], w["g5"], w["be5"])


# ---------------------------------------------------------------- scheduler

class _Gen:
    """A build-time pipeline: factory(pfx) -> generator yielding
    (engine, thunk) emission ops or ('stall', nsteps)."""

    def __init__(self, factory, after=(), ready_step=0, margin=2):
        self.factory = factory
        self.after = list(after)
        self.ready_step = ready_step
        self.margin = margin
        self.it = None
        self.slot = None
        self.done = False
        self.peeked = None
        self.resume_at = 0
        self.done_at = None


class _Sched:
    def __init__(self, slots=("p0_", "p1_")):
        self.pending = []
        self.active = []
        self.free_slots = list(slots)

    def add(self, gen):
        self.pending.append(gen)

    def _activate(self, j):
        while self.free_slots and self.pending:
            g = self.pending[0]
            if g.ready_step > j:
                break
            if any((not a.done) or (a.done_at is not None and
                                    j < a.done_at + g.margin)
                   for a in g.after):
                break
            self.pending.pop(0)
            g.slot = self.free_slots.pop(0)
            g.it = g.factory(g.slot)
            self.active.append(g)

    def step(self, j, budget):
        """Emit filler ops for step j; returns used-count dict."""
        self._activate(j)
        budget = dict(budget)
        used = {k: 0 for k in budget}
        progress = True
        while progress and any(v > 0 for v in budget.values()):
            progress = False
            for g in list(self.active):
                if g.resume_at > j:
                    continue
                op = g.peeked
                g.peeked = None
                if op is None:
                    try:
                        op = next(g.it)
                    except StopIteration:
                        g.done = True
                        g.done_at = j
                        self.active.remove(g)
                        self.free_slots.append(g.slot)
                        self._activate(j)
                        continue
                if op[0] == "stall":
                    g.resume_at = j + op[1]
                    continue
                eng, thunk = op
                if budget.get(eng, 0) <= 0:
                    g.peeked = op
                    continue
                thunk()
                budget[eng] -= 1
                used[eng] = used.get(eng, 0) + 1
                progress = True
        return used

    def exhausted(self):
        return not self.pending and not self.active


# ---------------------------------------------------------------- builder

class _KB:
    def __init__(self, S=S_FULL, BL=B // NCORES):
        self.S, self.BL = S, BL
        self.NTOK = S * BL
        self.CH = min(512, self.NTOK)          # token chunk for MLP phases
        self.NCH = self.NTOK // self.CH
        self.QS = self.CH // BL                # j-range per (b,j)-chunk
        self.NQ = S // self.QS
        self.nc = bacc.Bacc("TRN2", target_bir_lowering=False, debug=False,
                            num_devices=NCORES)
        self.vec_map = {}
        self._vec_cols = 0

    def _reg_vec(self, name, ntiles):
        self.vec_map[name] = (self._vec_cols, ntiles)
        self._vec_cols += ntiles

    def declare(self):
        nc = self.nc
        NT = self.NTOK
        self.d_xt = nc.dram_tensor("xt", [D, NT], BF16, kind="ExternalInput")
        wshapes = dict(W1=(D, H), Wvo=(H, H), W2=(H, H), W3=(H, L), W4=(L, H),
                       W5=(H, L), Wd1=(L, H), Wd2=(H, H), Wd3=(H, D),
                       Wm=(D, D))
        self.d_w = {k: nc.dram_tensor(k.lower() + "16", list(v), BF16,
                                      kind="ExternalInput")
                    for k, v in wshapes.items()}
        self.d_um = nc.dram_tensor("um16", [D, D], BF16,
                                   kind="ExternalInput")
        for nm, n in [("b1", 4), ("g1", 4), ("be1", 4), ("bvo", 4),
                      ("b2", 4), ("g2", 4), ("be2", 4),
                      ("b3", 1), ("g3", 1), ("be3", 1),
                      ("b4", 4), ("g4", 4), ("be4", 4),
                      ("b5", 1), ("g5", 1), ("be5", 1),
                      ("bd1", 4), ("gd1", 4), ("bed1", 4),
                      ("bd2", 4), ("gd2", 4), ("bed2", 4),
                      ("bd3", 6), ("bm", 6), ("z0", 1), ("lneps", 1)]:
            self._reg_vec(nm, n)
        self.d_vecs = nc.dram_tensor("vecs", [128, self._vec_cols], F32,
                                     kind="ExternalInput")
        self.d_id = nc.dram_tensor("id16", [128, 128], BF16,
                                   kind="ExternalInput")
        self.d_out = nc.dram_tensor("tok_loss", [1, NT], F32,
                                    kind="ExternalOutput")

    def vcol(self, name, t=0):
        s, n = self.vec_map[name]
        assert t < n
        return self.vecs_sb[:, s + t:s + t + 1]

    # ---- device helpers -------------------------------------------------
    def load_weight_tiles(self, pool, dram, K, M, dtype=BF16):
        nc = self.nc
        tiles = []
        for k in range(K // 128):
            t = pool.tile([128, M], dtype, tag=f"w_{dram.name}_{k}",
                          name=f"w_{dram.name}_{k}")
            nc.sync.dma_start(t[:], dram[k * 128:(k + 1) * 128, :])
            tiles.append(t)
        return tiles

    # ---- pipeline generators -------------------------------------------
    def layer_gen(self, pfx, in_aps, w_tiles, M_out, *, bias, ln=None,
                  relu=False, out_aps=None, out_tag=None):
        """Generator: out = [relu|id]( LN?( in @ W + b ) ), transposed."""
        nc, CH = self.nc, self.CH
        n_k, n_m = len(in_aps), M_out // 128
        if out_aps is None:
            out_aps = [self.tmp_pool.tile([128, CH], BF16,
                                          tag=f"{pfx}{out_tag}{m}",
                                          name=f"{pfx}{out_tag}{m}")[:]
                       for m in range(n_m)]
        if ln is None:
            for m in range(n_m):
                ps = self.cmp_pool.tile([128, CH], F32, tag=f"{pfx}ps",
                                        name=f"{pfx}ps")
                for k in range(n_k):
                    def mm(ps=ps, m=m, k=k):
                        nc.tensor.matmul(ps[:],
                                         w_tiles[k][:, m * 128:(m + 1) * 128],
                                         in_aps[k], start=(k == 0),
                                         stop=(k == n_k - 1))
                    yield ("pe", mm)
                def cp(ps=ps, m=m):
                    nc.scalar.activation(out_aps[m], ps[:],
                                         AF.Relu if relu else AF.Identity,
                                         bias=self.vcol(bias, m))
                yield ("act", cp)
            yield ("stall", 2)
            self._ret = out_aps
            return
        # LN path — stage-separated so no PE op ever waits on a freshly
        # emitted cross-engine dependency (which would block the in-order
        # PE queue and stall the recurrence behind it).
        g_nm, be_nm = ln
        st1 = self.st_pool.tile([1, CH], F32, tag="st1", name=f"{pfx}st1")
        st2 = self.st_pool.tile([1, CH], F32, tag="st2", name=f"{pfx}st2")
        ys, sqs = [], []
        for m in range(n_m):
            ps = self.cmp_pool.tile([128, CH], F32, tag=f"{pfx}ps",
                                    name=f"{pfx}ps")
            for k in range(n_k):
                def mm(ps=ps, m=m, k=k):
                    nc.tensor.matmul(ps[:],
                                     w_tiles[k][:, m * 128:(m + 1) * 128],
                                     in_aps[k], start=(k == 0),
                                     stop=(k == n_k - 1))
                yield ("pe", mm)
            y = self.tmp_pool.tile([128, CH], BF16, tag=f"{pfx}y{m % 2}",
                                   name=f"{pfx}y{m % 2}")
            ys.append(y)
            def cp(ps=ps, y=y, m=m):
                nc.scalar.activation(y[:], ps[:], AF.Identity,
                                     bias=self.vcol(bias, m))
            yield ("act", cp)
        yield ("stall", 1)
        for m in range(n_m):
            sq = self.tmp_pool.tile([128, CH], BF16,
                                    tag=f"{pfx}sq{m % 2}",
                                    name=f"{pfx}sq{m % 2}")
            sqs.append(sq)
            def sqf(y=ys[m], sq=sq):
                nc.vector.tensor_mul(sq[:], y[:], y[:])
            yield ("dve", sqf)
        yield ("stall", 1)
        for m in range(n_m):
            def s1(y=ys[m], m=m):
                nc.tensor.matmul(st1[:], self.ones1[:, 0:1], y[:],
                                 start=(m == 0), stop=(m == n_m - 1),
                                 skip_group_check=(m != n_m - 1))
            yield ("pe", s1)
            def s2(sq=sqs[m], m=m):
                nc.tensor.matmul(st2[:], self.ones1[:, 0:1], sq[:],
                                 start=(m == 0), stop=(m == n_m - 1),
                                 skip_group_check=(m != n_m - 1))
            yield ("pe", s2)
        yield ("stall", 1)
        inv_f = 1.0 / M_out
        mean = self.row_pool.tile([1, CH], F32, name=f"{pfx}mean",
                                  tag=f"{pfx}mean")[:]
        ra = self.row_pool.tile([1, CH], F32, name=f"{pfx}ra",
                                tag=f"{pfx}ra")[:]
        rb_ = self.row_pool.tile([1, CH], F32, name=f"{pfx}rbr",
                                 tag=f"{pfx}rbr")[:]
        def rowm():
            nc.vector.tensor_scalar_mul(mean, st1[:], inv_f)
            nc.vector.tensor_scalar_mul(ra, st2[:], inv_f)
            nc.vector.scalar_tensor_tensor(rb_, mean, -1.0, mean,
                                           ALU.mult, ALU.mult)  # -mean^2
            nc.vector.tensor_add(ra, ra, rb_)                   # var
        yield ("dve", rowm)
        def sdf():
            nc.scalar.activation(ra, ra, AF.Sqrt,
                                 bias=self.vcol("lneps")[0:1])   # sd
        yield ("act", sdf)
        rbf = self.tmp1_pool.tile([1, CH], BF16, tag=f"{pfx}rbf",
                                  name=f"{pfx}rbf")
        mbf = self.tmp1_pool.tile([1, CH], BF16, tag=f"{pfx}mbf",
                                  name=f"{pfx}mbf")
        def rowr():
            nc.vector.reciprocal(rb_, ra)                       # rstd
            nc.vector.tensor_mul(ra, mean, rb_)                 # mean*rstd
            nc.vector.tensor_copy(rbf[:], rb_)
            nc.vector.tensor_copy(mbf[:], ra)
        yield ("dve", rowr)
        rb = self.tmp1_pool.tile([128, CH], BF16, tag=f"{pfx}rb",
                                 name=f"{pfx}rb")
        mrb = self.tmp1_pool.tile([128, CH], BF16, tag=f"{pfx}mrb",
                                  name=f"{pfx}mrb")
        def bc1():
            nc.gpsimd.partition_broadcast(rb[:], rbf[:])
        yield ("pool", bc1)
        def bc2():
            nc.gpsimd.partition_broadcast(mrb[:], mbf[:])
        yield ("pool", bc2)
        yield ("stall", 2)
        for m in range(n_m):
            def ap1(m=m):
                nc.vector.tensor_mul(out_aps[m], ys[m][:], rb[:])
                nc.vector.tensor_sub(out_aps[m], out_aps[m], mrb[:])
            yield ("dve", ap1)
            def ap2(m=m):
                nc.scalar.activation(out_aps[m], out_aps[m],
                                     AF.Relu if relu else AF.Identity,
                                     bias=self.vcol(be_nm, m),
                                     scale=self.vcol(g_nm, m))
            yield ("act", ap2)
        yield ("stall", 2)
        self._ret = out_aps

    def encode_gen(self, pfx, in_aps, lat_out_ap, w1_tiles=None):
        w = self.w_sb
        yield from self.layer_gen(pfx, in_aps,
                                  w1_tiles or w["W1"], H, bias="b1",
                                  ln=("g1", "be1"), relu=True, out_tag="h")
        h = self._ret
        yield from self.layer_gen(pfx, h, w["Wvo"], H, bias="bvo",
                                  out_tag="a")
        a = self._ret
        yield from self.layer_gen(pfx, a, w["W2"], H, bias="b2",
                                  ln=("g2", "be2"), relu=True, out_tag="h")
        g = self._ret
        yield from self.layer_gen(pfx, g, w["W3"], L, bias="b3",
                                  ln=("g3", "be3"), out_tag="a")
        zl = self._ret
        yield from self.layer_gen(pfx, zl, w["W4"], H, bias="b4",
                                  ln=("g4", "be4"), relu=True, out_tag="h")
        e = self._ret
        yield from self.layer_gen(pfx, e, w["W5"], L, bias="b5",
                                  ln=("g5", "be5"), out_aps=[lat_out_ap])

    def sumsq_gen(self, pfx, a_aps, out_psum, wcol, b_aps=None):
        """out_psum[1,CH] = sum_over_partitions(a*b) per column."""
        nc = self.nc
        n = len(a_aps)
        for m in range(n):
            sq = self.tmp_pool.tile([128, self.CH], BF16, tag=f"{pfx}gsq",
                                    name=f"{pfx}gsq")
            o = b_aps[m] if b_aps is not None else a_aps[m]
            def sqf(m=m, sq=sq, o=o):
                nc.vector.tensor_mul(sq[:], a_aps[m], o)
            yield ("dve", sqf)
            def smm(m=m, sq=sq):
                nc.tensor.matmul(out_psum, wcol, sq[:],
                                 start=(m == 0), stop=(m == n - 1),
                                 skip_group_check=(m != n - 1))
            yield ("pe", smm)

    def decode_gen(self, pfx, c):
        """decode chunk c (b-major): recon+trans rows into SBUF."""
        nc, CH, S, BL = self.nc, self.CH, self.S, self.BL
        cs = slice(c * CH, (c + 1) * CH)
        yield from self.layer_gen(pfx, [self.latx[:, cs]], self.wd1, H,
                                  bias="bd1", ln=("gd1", "bed1"), relu=True,
                                  out_tag="a")
        h1 = self._ret
        yield from self.layer_gen(pfx, h1, self.wd2, H, bias="bd2",
                                  ln=("gd2", "bed2"), relu=True, out_tag="h")
        h2 = self._ret
        psr = self.st_pool.tile([1, CH], F32, tag="st1", name=f"{pfx}str")
        rs = []
        for m in range(6):
            ps = self.cmp_pool.tile([128, CH], F32, tag=f"{pfx}ps",
                                    name=f"{pfx}ps")
            for k in range(4):
                def mm(ps=ps, m=m, k=k):
                    nc.tensor.matmul(ps[:],
                                     self.wd3[k][:, m * 128:(m + 1) * 128],
                                     h2[k], start=(k == 0), stop=(k == 3))
                yield ("pe", mm)
            r = self.tmp_pool.tile([128, CH], BF16, tag=f"{pfx}sq{m % 2}",
                                   name=f"{pfx}r")
            rs.append(r)
            def df(ps=ps, r=r, m=m):
                nc.vector.scalar_tensor_tensor(
                    r[:], ps[:], self.vcol("bd3", m), self.xt[m][:, cs],
                    ALU.add, ALU.subtract)
                nc.vector.tensor_mul(r[:], r[:], r[:])
            yield ("dve", df)
            # r consumed 2 m-groups later (sq tags have 4 buffers), so the
            # stat MM's dep is at least 2 emission rounds old
            if m >= 2:
                def smm(r=rs[m - 2], m=m):
                    nc.tensor.matmul(psr[:], self.ones1[:, 0:1], r[:],
                                     start=(m == 2), stop=False,
                                     skip_group_check=True)
                yield ("pe", smm)
        yield ("stall", 1)
        for m in range(4, 6):
            def smm(r=rs[m], m=m):
                nc.tensor.matmul(psr[:], self.ones1[:, 0:1], r[:],
                                 start=False, stop=(m == 5),
                                 skip_group_check=(m != 5))
            yield ("pe", smm)
        yield ("stall", 1)
        def recf():
            nc.vector.tensor_scalar(self.rec_row[:, cs], psr[:],
                                    1.0 / D, 10.0, ALU.mult, ALU.min)
        yield ("dve", recf)
        # trans: dif chunk vs prev-token lat
        dif = self.tmp1_pool.tile([128, CH], BF16, tag=f"{pfx}dif",
                                  name=f"{pfx}dif")
        cst = c * CH
        def diff():
            if cst == 0:
                nc.vector.tensor_sub(dif[:, 1:CH],
                                     self.latx[:, 1:CH],
                                     self.latx[:, 0:CH - 1])
                nc.vector.tensor_sub(dif[:, 0:1], self.latx[:, 0:1],
                                     self.vcol("z0"))
            else:
                nc.vector.tensor_sub(dif[:], self.latx[:, cst:cst + CH],
                                     self.latx[:, cst - 1:cst + CH - 1])
            for b in range(BL):
                c0 = b * S
                if c0 > 0 and cst <= c0 <= cst + CH - 1:
                    nc.vector.tensor_sub(
                        dif[:, c0 - cst:c0 - cst + 1],
                        self.latx[:, c0:c0 + 1], self.vcol("z0"))
        yield ("dve", diff)
        pst = self.st_pool.tile([1, CH], F32, tag="st1", name=f"{pfx}stt")
        yield from self.sumsq_gen(pfx, [dif[:]], pst[:],
                                  self.ones1[:, 0:1])
        yield ("stall", 1)
        def trnf():
            nc.vector.tensor_scalar(self.trn_row[:, cs], pst[:],
                                    1.0 / L, 10.0, ALU.mult, ALU.min)
        yield ("dve", trnf)

    def _qview(self, row_ap, q):
        v = row_ap.rearrange("p (b s) -> p b s", b=self.BL)
        return v[:, :, q * self.QS:(q + 1) * self.QS]

    def encm_gen(self, pfx, q):
        ins = [self.memv[:, k, :, q * self.QS:(q + 1) * self.QS]
               for k in range(6)]
        latm_v = self.latmv[:, :, q * self.QS:(q + 1) * self.QS]
        yield from self.encode_gen(pfx, ins, latm_v)

    def ctx_gen(self, pfx, q):
        """ctx loss + final tok combine + output DMA for (b,j)-chunk q."""
        nc, CH = self.nc, self.CH
        lx = self.latxv[:, :, q * self.QS:(q + 1) * self.QS]
        lm = self.latmv[:, :, q * self.QS:(q + 1) * self.QS]
        pnx = self.st_pool.tile([1, CH], F32, tag="st1", name=f"{pfx}pnx")
        pnm = self.st_pool.tile([1, CH], F32, tag="st2", name=f"{pfx}pnm")
        pdt = self.st_pool.tile([1, CH], F32, tag="st1", name=f"{pfx}pdt")
        sqs = []
        for i, (aa, bb) in enumerate(((lx, lx), (lm, lm), (lx, lm))):
            sq = self.tmp_pool.tile([128, CH], BF16, tag=f"{pfx}sq{i % 2}",
                                    name=f"{pfx}gsq")
            sqs.append(sq)
            def sqf(sq=sq, aa=aa, bb=bb):
                nc.vector.tensor_mul(sq[:], aa, bb)
            yield ("dve", sqf)
        yield ("stall", 2)
        for ps_, sq in ((pnx, sqs[0]), (pnm, sqs[1]), (pdt, sqs[2])):
            def smm(sq=sq, ps_=ps_):
                nc.tensor.matmul(ps_[:], self.ones1[:, 0:1], sq[:],
                                 start=True, stop=True,
                                 skip_group_check=True)
            yield ("pe", smm)
        yield ("stall", 1)
        nxr = self.row_pool.tile([1, CH], F32, name=f"{pfx}mean",
                                 tag=f"{pfx}mean")[:]
        nmr = self.row_pool.tile([1, CH], F32, name=f"{pfx}ra",
                                 tag=f"{pfx}ra")[:]
        dot = self.row_pool.tile([1, CH], F32, name=f"{pfx}rbr",
                                 tag=f"{pfx}rbr")[:]
        def sqt():
            nc.scalar.activation(nxr, pnx[:], AF.Sqrt)
            nc.scalar.activation(nmr, pnm[:], AF.Sqrt)
        yield ("act", sqt)
        def rcp():
            nc.vector.tensor_scalar_max(nxr, nxr, NORM_EPS)
            nc.vector.tensor_scalar_max(nmr, nmr, NORM_EPS)
            nc.vector.reciprocal(nxr, nxr)
            nc.vector.reciprocal(nmr, nmr)
            nc.vector.tensor_mul(dot, pdt[:], nxr)
            nc.vector.tensor_mul(dot, dot, nmr)
            # ctx = clip(1 - cos, 0, 10)
            nc.vector.tensor_scalar(dot, dot, -1.0, 1.0, ALU.mult, ALU.add)
            nc.vector.tensor_scalar(dot, dot, 0.0, 10.0, ALU.max, ALU.min)
        yield ("dve", rcp)
        tokc = self.row_pool.tile([1, CH], F32, name=f"{pfx}tokc",
                                  tag=f"{pfx}mean")[:]
        def comb():
            rec_v = self._qview(self.rec_row, q)
            trn_v = self._qview(self.trn_row, q)
            tok_v = tokc.rearrange("p (b s) -> p b s", b=self.BL)
            nc.vector.scalar_tensor_tensor(
                tok_v, trn_v, 0.3, rec_v, ALU.mult, ALU.add)
            nc.vector.scalar_tensor_tensor(
                tok_v, dot, 0.3, tok_v, ALU.mult, ALU.add)
        yield ("dve", comb)
        def dma():
            dst = self._qview(self.d_out.ap(), q)
            src = tokc.rearrange("p (b s) -> p b s", b=self.BL)
            nc.sync.dma_start(dst, src)
        yield ("dma", dma)

    def pt_gen(self, pfx, q, with_tanh0=False):
        """pt = Wm^T x + bm for (b, j)-chunk q (written bf16 to ptw)."""
        nc = self.nc
        ins = [self.xtv[:, k, :, q * self.QS:(q + 1) * self.QS]
               for k in range(6)]
        for m in range(6):
            ps = self.cmp_pool.tile([128, self.CH], F32, tag=f"{pfx}ps",
                                    name=f"{pfx}ps")
            for k in range(6):
                def mm(ps=ps, m=m, k=k):
                    nc.tensor.matmul(ps[:],
                                     self.wm[k][:, m * 128:(m + 1) * 128],
                                     ins[k], start=(k == 0), stop=(k == 5))
                yield ("pe", mm)
            def cp(ps=ps, m=m):
                nc.scalar.activation(
                    self.ptv[:, m, :, q * self.QS:(q + 1) * self.QS],
                    ps[:], AF.Identity, bias=self.vcol("bm", m))
            yield ("act", cp)
        if with_tanh0:
            def t0():
                nc.scalar.activation(self.memv[:, :, :, 0],
                                     self.ptv[:, :, :, 0], AF.Tanh)
            yield ("act", t0)

    # ---- main build -----------------------------------------------------
    def build(self):
        nc = self.nc
        NT, CH, S, BL = self.NTOK, self.CH, self.S, self.BL
        self.declare()
        with tile.TileContext(nc) as tc:
            with (
                tc.tile_pool(name="const", bufs=1) as const_pool,
                tc.tile_pool(name="wenc", bufs=1) as wenc_pool,
                tc.tile_pool(name="big", bufs=1) as big_pool,
                tc.tile_pool(name="tmp", bufs=2) as tmp_pool,
                tc.tile_pool(name="tmp1", bufs=1) as tmp1_pool,
                tc.tile_pool(name="rows", bufs=1) as row_pool,
                tc.tile_pool(name="cmp", bufs=2, space="PSUM") as cmp_pool,
                tc.tile_pool(name="st", bufs=2, space="PSUM") as st_pool,
                tc.tile_pool(name="recps", bufs=2, space="PSUM") as rec_pool,
            ):
                self.tmp_pool, self.row_pool = tmp_pool, row_pool
                self.tmp1_pool = tmp1_pool
                self.cmp_pool, self.st_pool = cmp_pool, st_pool

                # constants
                self.ones1 = const_pool.tile([128, 1], BF16)
                nc.vector.memset(self.ones1[:], 1.0)
                self.vecs_sb = const_pool.tile([128, self._vec_cols], F32)
                nc.sync.dma_start(self.vecs_sb[:], self.d_vecs[:, :])

                # weights
                self.w_sb = {}
                for k, (K, M) in dict(W1=(D, H), Wvo=(H, H), W2=(H, H),
                                      W3=(H, L), W4=(L, H), W5=(H, L)).items():
                    self.w_sb[k] = self.load_weight_tiles(wenc_pool,
                                                          self.d_w[k], K, M)
                self.wd1 = self.load_weight_tiles(wenc_pool, self.d_w["Wd1"],
                                                  L, H)
                self.wd2 = self.load_weight_tiles(wenc_pool, self.d_w["Wd2"],
                                                  H, H)
                self.wd3 = self.load_weight_tiles(wenc_pool, self.d_w["Wd3"],
                                                  H, D)
                self.wm = self.load_weight_tiles(wenc_pool, self.d_w["Wm"],
                                                 D, D)
                self.um = self.load_weight_tiles(wenc_pool, self.d_um,
                                                 D, D)
                id_sb = wenc_pool.tile([128, 128], BF16, name="id_sb")
                nc.sync.dma_start(id_sb[:], self.d_id[:, :])

                # big persistent tensors
                xtw = big_pool.tile([128, 6 * NT], BF16, tag="xtw",
                                    name="xtw")
                self.xt = [xtw[:, k * NT:(k + 1) * NT] for k in range(6)]
                for k in range(6):
                    nc.sync.dma_start(self.xt[k],
                                      self.d_xt[k * 128:(k + 1) * 128, :])
                self.xtv = xtw[:].rearrange("p (m b s) -> p m b s", m=6, b=BL)
                ptw = big_pool.tile([128, 6 * NT], BF16, tag="ptw",
                                    name="ptw")
                self.ptv = ptw[:].rearrange("p (m b s) -> p m b s", m=6, b=BL)
                memw = big_pool.tile([128, 6 * NT], BF16, tag="memw",
                                     name="memw")
                self.memv = memw[:].rearrange("p (m b s) -> p m b s", m=6,
                                              b=BL)
                latx = big_pool.tile([128, NT], BF16, tag="latx", name="latx")
                latm = big_pool.tile([128, NT], BF16, tag="latm", name="latm")
                self.latx, self.latm = latx[:], latm[:]
                self.latxv = latx[:].rearrange("p (b s) -> p b s", b=BL)
                self.latmv = latm[:].rearrange("p (b s) -> p b s", b=BL)
                self.rec_row = big_pool.tile([1, NT], BF16, tag="rec",
                                             name="rec")[:]
                self.trn_row = big_pool.tile([1, NT], BF16, tag="trn",
                                             name="trn")[:]

                # ---- pt chunk 0 + tanh(step 0), emitted inline
                for eng, thunk in self.pt_gen("p0_", 0, with_tanh0=True):
                    if eng != "stall":
                        thunk()

                # ---- filler pipelines (pt chunks first: soft deadlines)
                sched = _Sched(slots=("p0_",))
                for q in range(1, self.NQ):
                    sched.add(_Gen(lambda pfx, q=q: self.pt_gen(pfx, q),
                                   margin=0))
                encx = []
                for c in range(self.NCH):
                    g = _Gen(lambda pfx, c=c: self.encode_gen(
                        pfx, [self.xt[k][:, c * CH:(c + 1) * CH]
                              for k in range(6)],
                        self.latx[:, c * CH:(c + 1) * CH]))
                    encx.append(g)
                dec = []
                for c in range(self.NCH):
                    g = _Gen(lambda pfx, c=c: self.decode_gen(pfx, c),
                             after=(encx[c],))
                    dec.append(g)
                encm, ctx = [], []
                for q in range(self.NQ):
                    g = _Gen(lambda pfx, q=q: self.encm_gen(pfx, q),
                             ready_step=min((q + 1) * self.QS + 2, S + 1))
                    encm.append(g)
                for q in range(self.NQ):
                    need = {id(encm[q]): encm[q]}
                    for b in range(BL):
                        for jj in (q * self.QS, (q + 1) * self.QS - 1):
                            dd = dec[(b * S + jj) // CH]
                            need[id(dd)] = dd
                    g = _Gen(lambda pfx, q=q: self.ctx_gen(pfx, q),
                             after=tuple(need.values()))
                    ctx.append(g)
                order = [encx[0]]
                if self.NCH > 2:
                    order.append(encx[2])
                order.append(dec[0])
                if self.NCH > 2:
                    order.append(dec[2])
                for c in range(self.NCH):
                    if encx[c] not in order:
                        order += [encx[c], dec[c]]
                def add_mlp_gens():
                    if int(_os.environ.get("KNOFILL", "0")):
                        return
                    for g in order:
                        sched.add(g)
                    for q in range(self.NQ):
                        sched.add(encm[q])
                        sched.add(ctx[q])

                add_mlp_gens()

                # ---- interleaved recurrence
                BUD_F = {"pe": 3, "act": 1, "dve": 3, "pool": 1, "dma": 1}

                # Split-tanh recurrence: psum halves A (m 0..2) and B
                # (m 3..5) close as separate accumulation groups, tanh'd by
                # two ACT ops.  Step j+1's matmuls are ordered k-ascending,
                # so its first 18 MMs depend only on tanh_A(j) — which
                # completed during step j's B-half MMs.  The serial
                # drain+sem+ACT+sem window hides behind real matmul work.
                HB = 3 * BL

                def rec_mm(cur, j, m, k, stop=False):
                    nc.tensor.matmul(
                        cur[:, m * BL:(m + 1) * BL],
                        self.um[k][:, m * 128:(m + 1) * 128],
                        self.memv[:, k, :, j - 1],
                        start=False, stop=stop, skip_group_check=True)

                def rec_ids(t, j):
                    nc.tensor.matmul(t[:], id_sb[:],
                                     self.ptv[:, :, :, j],
                                     start=True, stop=False,
                                     skip_group_check=True)

                cur = rec_pool.tile([128, 6 * BL], F32, tag="rps",
                                    name="rps")
                rec_ids(cur, 1)
                for j in range(1, S):
                    psv = cur[:].rearrange("p (m b) -> p m b", m=6)
                    # pass1: k 0..2 (deps: tanh_A of j-1), both halves
                    for k in range(3):
                        for m in range(6):
                            rec_mm(cur, j, m, k)
                    # pass2: k 3..5 for half A (m 0..2); close A
                    for k in range(3, 6):
                        for m in range(3):
                            rec_mm(cur, j, m, k, stop=(k == 5 and m == 2))
                    # A-half tanh runs while B still accumulates k 3..5
                    nc.scalar.activation(self.memv[:, 0:3, :, j],
                                         psv[:, 0:3, :], AF.Tanh)
                    # pass3: k 3..5 for half B (m 3..5); close B
                    for k in range(3, 6):
                        for m in range(3, 6):
                            rec_mm(cur, j, m, k, stop=(k == 5 and m == 5))
                    if j < S - 1:
                        nxt = rec_pool.tile([128, 6 * BL], F32, tag="rps",
                                            name="rps")
                        rec_ids(nxt, j + 1)
                    else:
                        nxt = None
                    nc.scalar.activation(self.memv[:, 3:6, :, j],
                                         psv[:, 3:6, :], AF.Tanh)
                    # fillers ride along with the recurrence stream
                    sched.step(j, BUD_F)
                    cur = nxt

                # ---- tail: drain remaining pipelines
                j = S
                BIG = {"pe": 64, "act": 16, "dve": 16, "pool": 4, "dma": 4}
                while not sched.exhausted():
                    sched.step(j, BIG)
                    j += 1
                    assert j < S + 4000, "filler pipelines did not drain"
        nc.compile()
        return nc


# ---------------------------------------------------------------- runner

_CACHE = {}


def _get_built(S, BL):
    key = (S, BL)
    if key not in _CACHE:
        kb = _KB(S, BL)
        kb.build()
        _CACHE[key] = kb
    return _CACHE[key]


def _host_inputs(kb, inputs):
    S, BL = kb.S, kb.BL
    w = {k: np.asarray(v, np.float32) for k, v in inputs.items()}
    Wvo = w["Wv"] @ w["Wo"]
    bvo = w["bv"] @ w["Wo"] + w["bo"]
    wd = dict(w)
    wd["Wvo"], wd["bvo"] = Wvo, bvo
    z0 = _encode_np(np.zeros((1, D), np.float32), wd)[0]

    vecs = _pack_cols(w["b1"], w["g1"], w["be1"], bvo,
                      w["b2"], w["g2"], w["be2"],
                      w["b3"], w["g3"], w["be3"],
                      w["b4"], w["g4"], w["be4"],
                      w["b5"], w["g5"], w["be5"],
                      w["bd1"], w["gd1"], w["bed1"],
                      w["bd2"], w["gd2"], w["bed2"],
                      w["bd3"], w["bm"], z0,
                      np.full(128, LN_EPS, np.float32))

    def b16(x):
        return np.ascontiguousarray(x.astype(ml_dtypes.bfloat16))

    def f8(x):
        return np.ascontiguousarray(x.astype(ml_dtypes.float8_e4m3))

    ident = np.eye(128, dtype=np.float32)
    shared = dict(id16=b16(ident),
                  w116=b16(w["W1"]), wvo16=b16(Wvo), w216=b16(w["W2"]),
                  w316=b16(w["W3"]), w416=b16(w["W4"]), w516=b16(w["W5"]),
                  wd116=b16(w["Wd1"]), wd216=b16(w["Wd2"]),
                  wd316=b16(w["Wd3"]), wm16=b16(w["Wm"]), um16=b16(w["Um"]),
                  vecs=vecs)

    seqs = np.asarray(inputs["sequences"], np.float32)
    in_maps = []
    for c in range(NCORES):
        xs = seqs[c * BL:(c + 1) * BL, :S, :]           # [BL, S, D]
        xt = b16(xs.reshape(BL * S, D).T)
        m = dict(shared)
        m["xt"] = xt
        in_maps.append(m)
    return in_maps


def _l2_term(inputs):
    names = ["W1", "b1", "g1", "be1", "Wv", "bv", "Wo", "bo", "W2", "b2", "g2",
             "be2", "W3", "b3", "g3", "be3", "W4", "b4", "g4", "be4", "W5",
             "b5", "g5", "be5", "Wd1", "bd1", "gd1", "bed1", "Wd2", "bd2",
             "gd2", "bed2", "Wd3", "bd3", "Wm", "Um", "bm"]
    l2 = sum(np.linalg.norm(np.asarray(inputs[n], np.float64)) for n in names)
    return float(np.clip(l2, 0.0, 10.0))


def _combine(kb, res, inputs):
    tok = np.concatenate([res.results[c]["tok_loss"].reshape(-1)
                          for c in range(NCORES)])
    l2 = _l2_term(inputs)
    per_tok = np.clip(tok.astype(np.float64) + 1e-4 * l2, 0.0, 100.0)
    nb = kb.BL * NCORES
    return np.float32(per_tok.sum() / nb)


def kernel(**inputs):
    seqs = np.asarray(inputs["sequences"])
    S = seqs.shape[1]
    BL = seqs.shape[0] // NCORES
    kb = _get_built(S, BL)
    in_maps = _host_inputs(kb, inputs)
    res = run_bass_kernel_spmd(kb.nc, in_maps, list(range(NCORES)))
    return _combine(kb, res, inputs)
